# revision 44
# baseline (speedup 1.0000x reference)
"""Trainium2 Bass kernel for nn_MetaMultiHeadSelfAttention_45810121179385.

Multi-head causal self-attention: B=4, S=2048, D=1024, H=16 heads (hd=64).

Sharding (8 NeuronCores): batch (4) x head-group (2 groups of 8 heads).
Core c handles batch b = c//2, head group g = c%2:
  - QKV projections for its 512 head-dims (tensor parallel on d_k rows)
  - attention for its 8 heads (full sequence, causal)
  - partial o_proj (columns of o_proj for its 512 v-dims)
Host sums the two partial outputs per batch and stacks batches.

v3: fully pipelined rounds. All operands bf16 (psum stays f32). The kernel
runs 4 rounds; round sc computes the QKV projections for sequence chunk
[512*sc, 512*sc+512) and then attention + o_proj for q-chunk qc=sc (which
only needs K/V up to that chunk — causal). PSUM budget: 2x [128,1024]
scores tiles (4 banks) + 2x [65,512] pv tiles (2 banks) + 2x [128,512]
projection/o_proj work tiles (2 banks) = 8 banks, so projection chains for
round sc+1 overlap attention of round sc and ScalarE (exp — the bottleneck
engine) starts ~25us into the kernel instead of after all projections.

v4 (this session):
  - ONE packed ExternalInput per core ([1024, 4224] bf16 holding x^T, the
    three projection weights, o_proj and the mask): per-call dispatch cost
    through the PJRT/axon execute path scales strongly with the number of
    argument buffers (measured ~80-100us per extra buffer per call), and
    device-side DMA volume is unchanged. Measured chained-slope time fell
    from ~726us to the ~470-590us band with this alone.
  - Slab-major round-0 projection chains (_proj_chains_prologue): all four
    chains of a kind in flight, each matmul gated only on its own 256KB
    slab DMA, cutting ~5us of prologue PE idle.
  - o_proj rounds 0..2 are deferred and emitted BETWEEN round-3 attention
    pairs (fillers, placed before each pair's normalize block): round 3 is
    ACT(exp)-bound (~26us PE deficit) and these are the only legal PE
    filler; in-order PE streams mean placement in the stream is what
    matters, and putting them after the normalize would strand them behind
    a DVE-chain stall.
  - o_proj(3) split: pairs 0-2 are contracted into an SBUF f32 partial
    before the last attention pair runs; only the pair-3 matmul + DVE add
    remain in the serial tail after the final normalize.
  - fp8 was evaluated and rejected on principle: iid per-element relative
    quantization error of a matmul operand passes through to the output
    Frobenius error at full strength (~4-6% for e4m3), over the 2e-2
    budget. bf16 everywhere stays (~0.6% measured).
  - TimelineSim: 270.4us -> 262.9us (PE busy 229us; the sim charges the
    row-group-paired QK matmuls serially, real PE is ~29us less, so the
    ACT-bound stretches and these fillers matter more on HW than in sim).

Attention is per head-PAIR: the two heads' QK^T matmuls (contraction hd=64)
land on PE row-groups 0-1 / 2-3 (auto tile_position (0,0)/(64,0)) and
execute concurrently in the systolic array. Scores for two consecutive
k-tiles share one [128,1024] psum tile so exp runs as one wide activation.
Softmax denominators come from a ones column appended to V inside the P@V
matmul (pv row 64).

Device layouts (per core):
  xT   [1024, 2048]  x[b] transposed (d on partitions), bf16
  wqT/wkT/wvT [1024, 512]  projection weights transposed, bf16
  woT  [512, 1024]   o_proj columns for this group, transposed, bf16
  yT   [1024, 2048]  partial output transposed [m, s], f32
"""

import functools
import os
import sys

import numpy as np

sys.path.insert(0, "/opt/trn_rl_repo")

import concourse.bass as bass  # noqa: E402
import concourse.tile as tile  # noqa: E402
from concourse import bacc, mybir  # noqa: E402
from concourse.bass_utils import run_bass_kernel_spmd  # noqa: E402

F32 = mybir.dt.float32
BF16 = mybir.dt.bfloat16
EXP = mybir.ActivationFunctionType.Exp

B, S, D, H, HD = 4, 2048, 1024, 16, 64
NCORES = 8
HPC = 8          # heads per core
GD = HPC * HD    # 512 head-dims per core
NKT = S // 128   # 16 kpos tiles
NQC = S // 512   # 4 q chunks of 512
NDC = D // 128   # 8 contraction chunks for projections
NVT = GD // 128  # 4 dk/v tiles per core = head pairs
SCALE = 1.0 / np.sqrt(HD)

DEFAULT_OPTS = {
    "e_bufs": 10,    # exp-output lookahead tiles
    "r_bufs": 3,
    "w_bufs": 2,     # shared projection/o_proj psum work tiles (1 bank each)
    "y_bufs": 6,
}
OPTS = dict(DEFAULT_OPTS)

VARIANTS = {
    "": {},
    "eb4": {"e_bufs": 4},
    "eb8": {"e_bufs": 8},
    "eb10": {"e_bufs": 10},
}


def _proj_chains_prologue(
    tc, x_sb, wq_sb, wk_sb, wv_sb, qt_sb, kt_sb, v_sb, wpool, ps_sc
):
    """Round-0 projection chains, slab-major: all four chains of a kind run
    concurrently, each matmul gated only on its own 256KB slab's DMA — the
    PE paces with DMA arrival instead of stalling a whole chain on the last
    slab. Two extra psum tiles are borrowed from the (still idle) score
    pool."""
    nc = tc.nc

    def quad(w_sb, out_sb, rhs_is_w=False):
        ps = [
            wpool.tile([128, 512], F32, tag="w", name=f"pp{id(w_sb)}_{t}")
            for t in range(2)
        ] + [
            ps_sc.tile([128, 1024], F32, tag="sc", name=f"pp{id(w_sb)}_{t}")[
                :, 0:512
            ]
            for t in range(2, 4)
        ]
        for k in range(NDC):
            for t in range(4):
                if rhs_is_w:
                    lhsT = x_sb[:, k, 128 * t : 128 * (t + 1)]
                    rhs = w_sb[:, k, :]
                else:
                    lhsT = w_sb[:, k, 128 * t : 128 * (t + 1)]
                    rhs = x_sb[:, k, 0:512]
                nc.tensor.matmul(
                    ps[t], lhsT=lhsT, rhs=rhs, start=(k == 0), stop=(k == NDC - 1)
                )
        for t in range(4):
            if rhs_is_w:
                nc.vector.tensor_copy(
                    out=v_sb[:, t, :, 0:HD],
                    in_=ps[t].rearrange("p (h d) -> p h d", h=HPC),
                )
            else:
                nc.vector.tensor_copy(out=out_sb[:, t, 0:512], in_=ps[t])

    quad(wq_sb, qt_sb)
    quad(wk_sb, kt_sb)
    quad(wv_sb, None, rhs_is_w=True)


def _proj_chains(tc, sc, x_sb, wq_sb, wk_sb, wv_sb, qt_sb, kt_sb, v_sb, wpool):
    """QKV projection chains for sequence chunk sc (cols 512*sc..+512)."""
    nc = tc.nc
    s0 = 512 * sc
    # pair-0's q/k first, then all v chains, then remaining q/k: attention
    # round sc (pair 0) becomes ready ~8us into the block, so ScalarE starts
    # on the round's exp work while the remaining chains still run
    for t in [0, None, 1, 2, 3]:
        if t is None:
            _v_chains(tc, sc, x_sb, wv_sb, v_sb, wpool)
            continue
        ps_q = wpool.tile([128, 512], F32, tag="w", name=f"psq{sc}_{t}")
        for k in range(NDC):
            nc.tensor.matmul(
                ps_q,
                lhsT=wq_sb[:, k, 128 * t : 128 * (t + 1)],
                rhs=x_sb[:, k, s0 : s0 + 512],
                start=(k == 0),
                stop=(k == NDC - 1),
            )
        nc.vector.tensor_copy(out=qt_sb[:, t, s0 : s0 + 512], in_=ps_q)
        ps_k = wpool.tile([128, 512], F32, tag="w", name=f"psk{sc}_{t}")
        for k in range(NDC):
            nc.tensor.matmul(
                ps_k,
                lhsT=wk_sb[:, k, 128 * t : 128 * (t + 1)],
                rhs=x_sb[:, k, s0 : s0 + 512],
                start=(k == 0),
                stop=(k == NDC - 1),
            )
        nc.vector.tensor_copy(out=kt_sb[:, t, s0 : s0 + 512], in_=ps_k)


def _v_chains(tc, sc, x_sb, wv_sb, v_sb, wpool):
    nc = tc.nc
    s0 = 512 * sc
    for vt in range(NVT):
        kti = 4 * sc + vt
        ps_v = wpool.tile([128, 512], F32, tag="w", name=f"psv{kti}")
        for k in range(NDC):
            nc.tensor.matmul(
                ps_v,
                lhsT=x_sb[:, k, s0 + 128 * vt : s0 + 128 * (vt + 1)],
                rhs=wv_sb[:, k, :],
                start=(k == 0),
                stop=(k == NDC - 1),
            )
        nc.vector.tensor_copy(
            out=v_sb[:, kti, :, 0:HD],
            in_=ps_v.rearrange("p (h d) -> p h d", h=HPC),
        )


def _attention_round(
    tc, qc, qt_sb, kt_sb, v_sb, ot_sb, mask_sb, rb_dram, epool, rpool, ps_sc,
    ps_pv, wpool=None, ones65=None, fillers=None, pre_last=None,
):
    """Attention for q-chunk qc over all head pairs (causal: kt <= 4*qc+3).

    fillers: optional list of callables emitting PE filler work (o_proj
    m-chunks); a share is emitted after each pair so the in-order PE stream
    has runnable matmuls during the ACT(exp)-bound stretches."""
    nc = tc.nc
    q0 = 512 * qc
    ktn = 4 * (qc + 1)
    last_kt = ktn - 1
    for p in range(NVT):
        if p == NVT - 1 and pre_last is not None:
            pre_last()
        pv = [
            ps_pv.tile([65, 512], F32, tag="pv", name=f"pv{p}_{qc}_{par}")
            for par in range(2)
        ]
        pvd_t = [
            rpool.tile([65, 512], BF16, tag="pvd", name=f"pvd{p}_{qc}_{par}")
            for par in range(2)
        ]
        for kt in range(ktn):
            off = max(0, 128 * kt - q0)
            # scores^T[k, q]: BOTH parities of the pair into ONE psum slot
            # (par0 cols 0:512, par1 cols 512:1024). The two matmuls share
            # the slot dependency so they become ready together and schedule
            # back-to-back — their disjoint PE row-groups (0-1 vs 2-3, from
            # lhsT base partition 0/64) then overlap in the systolic array.
            sc_t = ps_sc.tile([128, 1024], F32, tag="sc", name=f"sc{p}_{qc}_{kt}")
            for par in range(2):
                p_h = 64 * par
                c0 = off if par == 0 else 512  # par1 packed flush at 512
                nc.tensor.matmul(
                    sc_t[:, c0 : c0 + 512 - off],
                    lhsT=kt_sb[p_h : p_h + 64, p, 128 * kt : 128 * kt + 128],
                    rhs=qt_sb[p_h : p_h + 64, p, q0 + off : q0 + 512],
                    start=True,
                    stop=True,
                )
            # one wide exp covering both parities contiguously (elementwise —
            # the halves being different heads doesn't matter)
            e_t = epool.tile([128, 1024], BF16, tag="e", name=f"e{p}_{qc}_{kt}")
            nc.scalar.activation(
                out=e_t[:, off : 1024 - off],
                in_=sc_t[:, off : 1024 - off],
                func=EXP,
                scale=SCALE,
            )
            # causal mask on diagonal 128x128 blocks. High priority: the
            # multiply is on the exp->PV critical chain and must beat the
            # (off-critical-path) normalize work to the DVE queue. Both
            # parities' blocks (cols off and 512) ride in ONE op via a 3D
            # AP with a stride-0 broadcast on the mask operand — halves the
            # op count and the per-op overhead on the critical chain.
            if 128 * kt >= q0:
                e0 = e_t[:, off : off + 128]
                e2 = bass.AP(
                    tensor=e0.tensor,
                    offset=e0.offset,
                    ap=[list(e0.ap[0]), [512 - off, 2], list(e0.ap[1])],
                )
                m2 = bass.AP(
                    tensor=mask_sb.tensor,
                    offset=mask_sb.offset,
                    ap=[list(mask_sb.ap[0]), [0, 2], list(mask_sb.ap[1])],
                )
                with tc.high_priority():
                    nc.vector.tensor_mul(e2, e2, m2)
            # PV accumulation (+ denominator via ones column)
            for par in range(2):
                c0 = off if par == 0 else 512
                nc.tensor.matmul(
                    pv[par][:, off:512],
                    lhsT=v_sb[:, kt, 2 * p + par, :],
                    rhs=e_t[:, c0 : c0 + 512 - off],
                    start=(kt == 0),
                    stop=(kt == last_kt),
                )
        # PE fillers go in the stream BEFORE the normalize: the last pair's
        # normalize chain (DVE) stalls the PE at the broadcast matmul, and
        # in-order execution would strand ready filler work behind it.
        if fillers:
            share = max(1, (len(fillers) + NVT - 1 - p) // (NVT - p))
            for f in [fillers.pop(0) for _ in range(min(share, len(fillers)))]:
                f()
        if wpool is not None and p == NVT - 1 and ones65 is not None:
            # HAM insurance: dep-free junk matmuls execute during the last
            # normalize stall (2.5-3.2us PE gaps in sim, just under the
            # ~3.4us MID window that would re-throttle the PE to 1.2GHz on
            # HW and slow the whole o_proj tail).
            wm = ps_sc.tile([128, 1024], F32, tag="sc", name="warm_tail")
            for _ in range(30):
                nc.tensor.matmul(
                    wm[0:64, 0:64],
                    lhsT=ones65[0:64, :],
                    rhs=ones65[0:64, :],
                    start=True,
                    stop=True,
                )
        if wpool is not None and p == NVT - 1:
            # very last pair: latency-optimized normalize. Reciprocals read
            # the pv psum row directly (no pvd-copy dependency) so the
            # PE broadcasts fire early, and par1's chain — whose output must
            # cross partitions via the st_t DMA, the tail's long pole —
            # runs before par0's so the DMA overlaps par0's normalize.
            rb_ts = []
            for par in range(2):
                r_t = rpool.tile([65, 512], F32, tag="r", name=f"rl{p}_{par}")
                nc.vector.reciprocal(out=r_t[64:65, :], in_=pv[par][64:65, :])
                r_bf = rpool.tile([65, 512], BF16, tag="rbf", name=f"rbf{p}_{par}")
                nc.vector.tensor_copy(out=r_bf[64:65, :], in_=r_t[64:65, :])
                rb_ps = wpool.tile([128, 512], F32, tag="w", name=f"rbp{p}_{par}")
                nc.tensor.matmul(
                    rb_ps[0:64, :],
                    lhsT=ones65[64:65, :],
                    rhs=r_bf[64:65, :],
                    start=True,
                    stop=True,
                )
                rb_ts.append(rb_ps[0:64, :])
            nc.vector.tensor_copy(out=pvd_t[1], in_=pv[1])
            st_t = rpool.tile([64, 512], BF16, tag="st", name=f"st{p}_{qc}")
            nc.vector.tensor_mul(st_t, pvd_t[1][0:64, :], rb_ts[1])
            nc.sync.dma_start(out=ot_sb[64:128, p, q0 : q0 + 512], in_=st_t)
            nc.vector.tensor_copy(out=pvd_t[0], in_=pv[0])
            nc.vector.tensor_mul(
                ot_sb[0:64, p, q0 : q0 + 512], pvd_t[0][0:64, :], rb_ts[0]
            )
            continue
        # normalize: divide by the ones-column row (row 64). Copy pv out of
        # PSUM first (single DVE op) so the banks free for the next pair
        # immediately; the reciprocal + DRAM broadcast + multiply chain runs
        # off the critical path.
        for par in range(2):
            h = 2 * p + par
            pvd = pvd_t[par]
            nc.vector.tensor_copy(out=pvd, in_=pv[par])
            r_t = rpool.tile([65, 512], F32, tag="r", name=f"r{p}_{qc}_{par}")
            nc.vector.reciprocal(out=r_t[64:65, :], in_=pvd[64:65, :])
            if wpool is not None:
                # final round: broadcast the reciprocal row across partitions
                # with a K=1 ones matmul instead of the DRAM round-trip — the
                # o_proj tail is waiting on this chain.
                r_bf = rpool.tile([65, 512], BF16, tag="rbf", name=f"rbf{p}_{par}")
                nc.vector.tensor_copy(out=r_bf[64:65, :], in_=r_t[64:65, :])
                rb_ps = wpool.tile([128, 512], F32, tag="w", name=f"rbp{p}_{par}")
                nc.tensor.matmul(
                    rb_ps[0:64, :],
                    lhsT=ones65[64:65, :],
                    rhs=r_bf[64:65, :],
                    start=True,
                    stop=True,
                )
                rb_t = rb_ps[0:64, :]
            else:
                nc.sync.dma_start(out=rb_dram[h, qc, :], in_=r_t[64:65, :])
                rb_t = rpool.tile([64, 512], F32, tag="rb", name=f"rb{p}_{qc}_{par}")
                src = rb_dram[h, qc, :]
                nc.sync.dma_start(
                    out=rb_t,
                    in_=bass.AP(
                        tensor=src.tensor,
                        offset=src.offset,
                        ap=[[0, 64]] + list(src.ap),
                    ),
                )
            if par == 0:
                nc.vector.tensor_mul(
                    ot_sb[0:64, p, q0 : q0 + 512], pvd[0:64, :], rb_t
                )
            else:
                st_t = rpool.tile([64, 512], BF16, tag="st", name=f"st{p}_{qc}")
                nc.vector.tensor_mul(st_t, pvd[0:64, :], rb_t)
                nc.sync.dma_start(
                    out=ot_sb[64:128, p, q0 : q0 + 512], in_=st_t
                )


def _oproj_mchunk(tc, qc, m, wo_sb, ot_sb, yT, wpool, ypool):
    nc = tc.nc
    q0 = 512 * qc
    y_sb = ypool.tile([128, 512], F32, tag="y", name=f"y{qc}_{m}")
    ps_y = wpool.tile([128, 512], F32, tag="w", name=f"psy{qc}_{m}")
    for t in range(NVT):
        nc.tensor.matmul(
            ps_y,
            lhsT=wo_sb[:, t, 128 * m : 128 * (m + 1)],
            rhs=ot_sb[:, t, q0 : q0 + 512],
            start=(t == 0),
            stop=(t == NVT - 1),
        )
    nc.vector.tensor_copy(out=y_sb, in_=ps_y)
    nc.sync.dma_start(
        out=yT[128 * m : 128 * (m + 1), q0 : q0 + 512], in_=y_sb
    )


def _oproj_round(tc, qc, wo_sb, ot_sb, yT, wpool, ypool):
    for m in range(D // 128):
        _oproj_mchunk(tc, qc, m, wo_sb, ot_sb, yT, wpool, ypool)


_KERNEL_INSTANCES = [0]


def _mha_tile_kernel(tc, xT, wqT, wkT, wvT, woT, mask, yT):
    nc = tc.nc
    _KERNEL_INSTANCES[0] += 1
    rb_dram = nc.dram_tensor(
        f"rb_dram{_KERNEL_INSTANCES[0]}", [HPC, NQC, 512], F32
    ).ap()
    with (
        tc.tile_pool(name="big", bufs=1) as big,
        tc.tile_pool(name="expT", bufs=OPTS["e_bufs"]) as epool,
        tc.tile_pool(name="rtiles", bufs=OPTS["r_bufs"]) as rpool,
        tc.tile_pool(name="ysb", bufs=OPTS["y_bufs"]) as ypool,
        tc.tile_pool(name="ps_w", bufs=OPTS["w_bufs"], space="PSUM") as wpool,
        tc.tile_pool(name="ps_sc", bufs=2, space="PSUM") as ps_sc,
        tc.tile_pool(name="ps_pv", bufs=2, space="PSUM") as ps_pv,
    ):
        wo_sb = big.tile([128, NVT, D], BF16, tag="wo")
        mask_sb = big.tile([128, 128], BF16, tag="mask")
        qt_sb = big.tile([128, NVT, S], BF16, tag="qt")   # Q^T [dk, s]
        kt_sb = big.tile([128, NVT, S], BF16, tag="kt")   # K^T [dk, s]
        # V in [kpos, dv] layout, 65 cols per head (64 data + ones col)
        v_sb = big.tile([128, NKT, HPC, HD + 1], BF16, tag="v")
        x_sb = big.tile([128, NDC, S], BF16, tag="x")     # full x^T resident
        wq_sb = big.tile([128, NDC, GD], BF16, tag="wq")
        wk_sb = big.tile([128, NDC, GD], BF16, tag="wk")
        wv_sb = big.tile([128, NDC, GD], BF16, tag="wv")
        ones65 = big.tile([65, 64], BF16, tag="ones65")
        nc.vector.memset(ones65, 1.0)
        # all 16x8 ones columns of V in one strided memset
        nc.vector.memset(v_sb[:, :, :, HD : HD + 1], 1.0)
        # warm-up while the prologue DMAs stream in:
        #  - junk matmuls keep the PE busy so the HAM clock-gate reaches
        #    8/8 (2.4 GHz) before the first real projection chain
        #  - a tiny exp preloads the ACT table set (~1.3us) off the
        #    critical path of the first attention round
        warm = ps_sc.tile([128, 1024], F32, tag="sc", name="warm")
        for _ in range(72):
            nc.tensor.matmul(
                warm[0:64, 0:64],
                lhsT=ones65[0:64, :],
                rhs=ones65[0:64, :],
                start=True,
                stop=True,
            )
        warme = rpool.tile([65, 512], F32, tag="r", name="warme")
        nc.scalar.activation(
            out=warme[0:1, 0:8], in_=ones65[0:1, 0:8], func=EXP, scale=1.0
        )

        # ---- DMA prologue: first matmul needs wq + x[0] only ----
        wqT_r = wqT.rearrange("(k p) g -> p k g", p=128)
        xT_r = xT.rearrange("(k p) s -> p k s", p=128)
        for lo, hi in ((0, 1), (1, 2), (2, 4), (4, 8)):
            nc.sync.dma_start(out=wq_sb[:, lo:hi, :], in_=wqT_r[:, lo:hi, :])
            nc.sync.dma_start(
                out=x_sb[:, lo:hi, 0:512], in_=xT_r[:, lo:hi, 0:512]
            )
        nc.sync.dma_start(out=wk_sb, in_=wkT.rearrange("(k p) g -> p k g", p=128))
        nc.sync.dma_start(out=wv_sb, in_=wvT.rearrange("(k p) g -> p k g", p=128))
        nc.sync.dma_start(out=mask_sb, in_=mask)
        # x[1:2] merged into one DMA: 2KB-per-partition lines (better DMA
        # granularity); completes ~15us in, well before block 1 needs it
        nc.sync.dma_start(out=x_sb[:, :, 512:1536], in_=xT_r[:, :, 512:1536])
        # wo packed as two row-halves of a [1024, 512] region:
        # rows 0:512 hold woT[:, 0:512], rows 512:1024 hold woT[:, 512:1024]
        nc.sync.dma_start(
            out=wo_sb[:, :, 0:512],
            in_=woT[0:GD, :].rearrange("(t p) m -> p t m", p=128),
        )
        nc.sync.dma_start(
            out=wo_sb[:, :, 512:1024],
            in_=woT[GD : 2 * GD, :].rearrange("(t p) m -> p t m", p=128),
        )
        nc.sync.dma_start(out=x_sb[:, :, 1536:2048], in_=xT_r[:, :, 1536:2048])

        with tc.tile_pool(name="outT", bufs=1) as opool:
            ot_sb = opool.tile([128, NVT, S], BF16, tag="ot")  # attn out^T
            # o_proj(3) partial accumulator (pairs 0..2), f32
            y3p_sb = opool.tile([128, D // 128, 512], F32, tag="y3p")
            for sc in range(NQC):
                if sc == 0:
                    _proj_chains_prologue(
                        tc, x_sb, wq_sb, wk_sb, wv_sb, qt_sb, kt_sb, v_sb,
                        wpool, ps_sc,
                    )
                else:
                    _proj_chains(
                        tc, sc, x_sb, wq_sb, wk_sb, wv_sb, qt_sb, kt_sb, v_sb,
                        wpool,
                    )
                # o_proj is deferred into the LAST round: round 3's attention
                # is ACT(exp)-bound with ~26us of PE idle, and o_proj rounds
                # 0-2 are the only legal filler there. Emitting them between
                # round-3 pairs puts runnable matmuls in the in-order PE
                # stream exactly where the exp waits happen. Rounds 1-2 are
                # filled by the next round's projection chains already.
                fillers = None
                pre_last = None
                if sc == NQC - 2:
                    # on real HW the paired-QK concurrency (which the sim
                    # charges serially) leaves round 2 ~3-4us short of PE
                    # filler — give it the first half of o_proj(0)
                    fillers = [
                        (lambda m=m: _oproj_mchunk(
                            tc, 0, m, wo_sb, ot_sb, yT, wpool, ypool
                        ))
                        for m in range(4)
                    ]
                if sc == NQC - 1:
                    fillers = [
                        (lambda qc=qc, m=m: _oproj_mchunk(
                            tc, qc, m, wo_sb, ot_sb, yT, wpool, ypool
                        ))
                        for qc in range(NQC - 1)
                        for m in range(D // 128)
                        if not (qc == 0 and m < 4)
                    ]
                    q3 = 512 * (NQC - 1)

                    def pre_last():
                        # o_proj(3) partial over pairs 0..2 into SBUF f32 —
                        # only the pair-3 matmul + add remain after the last
                        # normalize, cutting the serial tail.
                        for m in range(D // 128):
                            ps = wpool.tile(
                                [128, 512], F32, tag="w", name=f"p3a{m}"
                            )
                            for t in range(NVT - 1):
                                nc.tensor.matmul(
                                    ps,
                                    lhsT=wo_sb[:, t, 128 * m : 128 * (m + 1)],
                                    rhs=ot_sb[:, t, q3 : q3 + 512],
                                    start=(t == 0),
                                    stop=(t == NVT - 2),
                                )
                            nc.vector.tensor_copy(out=y3p_sb[:, m, :], in_=ps)

                _attention_round(
                    tc, sc, qt_sb, kt_sb, v_sb, ot_sb, mask_sb, rb_dram,
                    epool, rpool, ps_sc, ps_pv,
                    wpool=(wpool if sc == NQC - 1 else None),
                    ones65=ones65,
                    fillers=fillers,
                    pre_last=pre_last,
                )
            # o_proj(3) tail: add the pair-3 contribution to the partials.
            # Adds alternate DVE/Pool (both idle here) and y DMAs alternate
            # SP/ACT queues so no single engine paces the 8-chunk drain.
            q3 = 512 * (NQC - 1)
            t3 = NVT - 1
            for m in range(D // 128):
                ps = wpool.tile([128, 512], F32, tag="w", name=f"p3b{m}")
                nc.tensor.matmul(
                    ps,
                    lhsT=wo_sb[:, t3, 128 * m : 128 * (m + 1)],
                    rhs=ot_sb[:, t3, q3 : q3 + 512],
                    start=True,
                    stop=True,
                )
                y_sb = ypool.tile([128, 512], F32, tag="y", name=f"y3_{m}")
                nc.vector.tensor_add(y_sb, y3p_sb[:, m, :], ps)
                nc.sync.dma_start(
                    out=yT[128 * m : 128 * (m + 1), q3 : q3 + 512], in_=y_sb
                )


# packed input layout (one ExternalInput per core — per-call dispatch cost
# under the PJRT/axon execute path scales with the number of argument
# buffers, so all six logical inputs ride in one [D, PKC] bf16 tensor):
#   cols 0:2048       xT   [1024, 2048]
#   cols 2048:2560    wqT  [1024, 512]
#   cols 2560:3072    wkT  [1024, 512]
#   cols 3072:3584    wvT  [1024, 512]
#   cols 3584:4096    woT  rows 0:512 = woT[:, 0:512], rows 512:1024 = woT[:, 512:1024]
#   cols 4096:4224    mask [128, 128] in rows 0:128
PKC = S + 3 * GD + 512 + 128


@functools.lru_cache(maxsize=8)
def build_program(variant=None):
    if variant is None:
        variant = os.environ.get("MHA_VARIANT", "")
    OPTS.clear()
    OPTS.update(DEFAULT_OPTS)
    OPTS.update(VARIANTS[variant])
    nc = bacc.Bacc("TRN2", target_bir_lowering=False, debug=False)
    pk = nc.dram_tensor("pk", [D, PKC], BF16, kind="ExternalInput").ap()
    xT = pk[:, 0:S]
    wqT = pk[:, S : S + GD]
    wkT = pk[:, S + GD : S + 2 * GD]
    wvT = pk[:, S + 2 * GD : S + 3 * GD]
    woT = pk[:, S + 3 * GD : S + 3 * GD + 512]  # [1024, 512], two row-halves
    mask = pk[0:128, S + 3 * GD + 512 : S + 3 * GD + 512 + 128]
    # NB: keep the output f32 — a bf16 ExternalOutput measured ~5x slower
    # through the PJRT/axon execute path (slow output handling), despite the
    # smaller device-side DMA.
    yT = nc.dram_tensor("yT", [D, S], F32, kind="ExternalOutput").ap()
    with tile.TileContext(nc) as tc:
        _mha_tile_kernel(tc, xT, wqT, wkT, wvT, woT, mask, yT)
    nc.compile()
    return nc


def make_in_maps(x, q_proj, k_proj, v_proj, o_proj):
    import ml_dtypes

    bf = ml_dtypes.bfloat16
    x = np.asarray(x, dtype=np.float32)
    mask = np.triu(np.ones((128, 128), dtype=bf))  # keep iff col >= row
    in_maps = []
    for c in range(NCORES):
        b, g = divmod(c, 2)
        sl = slice(GD * g, GD * (g + 1))
        pk = np.empty((D, PKC), dtype=bf)
        pk[:, 0:S] = x[b].T
        pk[:, S : S + GD] = np.asarray(q_proj)[sl, :].T
        pk[:, S + GD : S + 2 * GD] = np.asarray(k_proj)[sl, :].T
        pk[:, S + 2 * GD : S + 3 * GD] = np.asarray(v_proj)[sl, :].T
        wo = np.asarray(o_proj)[:, sl].T  # [GD, D]
        pk[0:GD, S + 3 * GD : S + 3 * GD + 512] = wo[:, 0:512]
        pk[GD : 2 * GD, S + 3 * GD : S + 3 * GD + 512] = wo[:, 512:1024]
        pk[0:128, S + 3 * GD + 512 : S + 3 * GD + 512 + 128] = mask
        in_maps.append({"pk": pk})
    return in_maps


def gather_output(results):
    outs = [np.asarray(r["yT"], dtype=np.float32) for r in results]
    return np.stack(
        [(outs[2 * b] + outs[2 * b + 1]).T for b in range(B)], axis=0
    )


def kernel(x, q_proj, k_proj, v_proj, o_proj, _trace=False, _trace_kwargs=None):
    nc = build_program()
    in_maps = make_in_maps(x, q_proj, k_proj, v_proj, o_proj)
    res = run_bass_kernel_spmd(
        nc,
        in_maps,
        core_ids=list(range(NCORES)),
        trace=_trace,
        **(_trace_kwargs or {}),
    )
    y = gather_output(res.results)
    if _trace:
        kernel.last_result = res
    return y

